# revision 2
# baseline (speedup 1.0000x reference)
"""Trainium2 Bass kernel for nn_Attention (B=4, T=1024, C=1024, 16 heads).

Sharding: 8 cores = (batch b, head-group g). Core i = 2b+g handles heads
[8g, 8g+8) of batch b for ALL 1024 tokens — tensor-parallel over heads,
column-parallel Wproj. Zero redundant FLOPs (512 matmuls/core). Each
core emits the partial y^T = Wproj[:, cols g] @ out^T_g for all tokens;
the host unshard sums the two partials per batch.

Everything on-chip is laid out so no transposes are ever needed:
  - the host passes x^T; Q^T/K^T come out of their projections in
    [chan, tok] layout directly
  - V is produced in [tok, chan] layout into 128-wide per-head blocks:
    ones in col 0 (so the PV matmul's PSUM row 0 is the softmax
    denominator Z), v dims in cols 64:128 (so the PV output rows sit
    64-aligned for the normalization multiply)
  - softmax runs on S^T (keys on partitions) with no max-subtraction
    (logits are O(6), exp is safe); one [128,1024] exp covers both
    query halves of a head

Schedule (v2): the EXP stream (64 [128,1024] EXPs, 1.10us each on
ScalarE) paces the S blocks: S-pair(j) is gated on EXP(j-2) through the
2-buffer ps_s rotation, and every dependency-gated matmul that reaches
the PE queue head before its semaphore posts pays ~100ns (SEM_DELAY).
The fix is slack: every S block gets an explicit per-chunk filler
generator of independent matmuls so gated work never sits at the head:
  pair 0: V-projection chain (as before)
  pair 1: S_a <- kt2 proj; S_b <- qt2 proj + Y-kc0 partials
  pair 2: S_a <- kt3 proj; S_b <- qt3 proj + PV(4,*) interleaved
  pair 3: S_a <- Y-(kc1+kc2) chains m0-3; S_b <- m4,5 + PV(6,*)
The output projection is accumulated across pairs in SBUF:
part[m] = bias + kc0 (pair 1, DVE tensor_scalar_add with the [128,1]
bias column) += (kc1+kc2 psum chain) (pair 3, DVE add); the final Y
stage is only the kc3 matmuls plus one whole-tile DVE add (psum + part
-> bf16 ysb) and the store — no ScalarE in the output path at all, and
the old 14 warm-up filler matmuls in pair 3 are replaced by real work.

All matmul operands bf16 (fp8 would double PE throughput but its ~4%
quantization noise fails the 2e-2 gate); PSUM accumulation fp32; Y
partials bf16 in SBUF; y^T partials stored bf16 and summed on the host
in fp32.

Note: the part runs in one of two ambient clock modes (~2.37 GHz /
~1.98 GHz matmul streaming, switching on a tens-of-minutes timescale,
not kernel-controlled) — wall-ns comparisons across runs must be
normalized by the median matmul slice duration (216ns vs 259ns).
"""

import os

import numpy as np

USE_F32R = os.environ.get("KERNEL_F32R", "0") == "1"

B, T, C = 4, 1024, 1024
NH, HD = 16, 64
NHL = NH // 2          # heads per core (local)
GC = NHL * HD          # channels per head-group = 512
KC = C // 128          # 8 contraction chunks over full C
KCL = GC // 128        # 4 contraction chunks over the local half
SCALE = 1.0 / float(np.sqrt(HD))

_PROG = None


def _build_program():
    import concourse.bacc as bacc
    import concourse.mybir as mybir
    import concourse.tile as tile

    F32 = mybir.dt.float32
    BF16 = mybir.dt.float32r if USE_F32R else mybir.dt.bfloat16
    OBF16 = mybir.dt.bfloat16  # y^T partials: halves the output DMA tail
    Exp = mybir.ActivationFunctionType.Exp

    nc = bacc.Bacc()
    xt_d = nc.declare_dram_parameter("xt", [KC, 128, T], BF16, isOutput=False)
    wq_d = nc.declare_dram_parameter("wq", [KCL, 128, C], BF16, isOutput=False)
    wk_d = nc.declare_dram_parameter("wk", [KCL, 128, C], BF16, isOutput=False)
    wv_d = nc.declare_dram_parameter("wv", [KC, 128, GC], BF16, isOutput=False)
    wp_d = nc.declare_dram_parameter("wp", [KCL, 128, C], BF16, isOutput=False)
    bias_d = nc.declare_dram_parameter("bias", [128, 8], F32, isOutput=False)
    yt_d = nc.declare_dram_parameter("yt", [8, 128, T], OBF16, isOutput=True)

    from contextlib import ExitStack

    with ExitStack() as ctx:
        tc = ctx.enter_context(tile.TileContext(nc))
        ctx.enter_context(
            nc.allow_low_precision("bf16 matmul operands are intentional")
        )
        pool = lambda name, bufs, **kw: ctx.enter_context(  # noqa: E731
            tc.tile_pool(name=name, bufs=bufs, **kw)
        )
        xt_pool = pool("xt", KC)
        wqk_pool = pool("wqk", 4)
        wv_pool = pool("wv", KC)
        wp_pool = pool("wp", KCL)
        kt_pool = pool("kt", 2)
        qt_pool = pool("qt", 2)
        v_pool = pool("vaug", KC)
        exp_pool = pool("exp", 18)
        ot_pool = pool("ot", KCL)
        part_pool = pool("part", 8)  # Y partial sums, bf16 [128,T]
        y_pool = pool("ysb", 3)
        r_pool = pool("rsb", 2)
        rbx_pool = pool("rbx", 2)
        misc_pool = pool("misc", 2)
        ps_s = pool("pss", 2, space="PSUM")         # [128,1024] = 2 banks
        ps_small = pool("pssm", 4, space="PSUM")    # [128,512] = 1 bank

        # ---- stage 0: memset ones + streamed loads ------------------
        # DMA priority order = data-plane order (one FIFO at ~436 GB/s):
        # kw0 and the x^T first halves gate the first projection chain,
        # so qw0 (needed ~2us later) and bias (needed only at Y) queue
        # behind them.
        ones_sb = misc_pool.tile([128, 528], BF16, tag="ones", name="ones_sb")
        nc.gpsimd.memset(ones_sb[:], 1.0)

        # x^T loads split by column half, all first halves ahead of the
        # second: the kt0/qt0 n=0 psum chains only read columns 0:512,
        # so they start ~2x earlier than with whole-tile loads.
        kw = {0: wqk_pool.tile([128, C], BF16, tag="wqk", name="kw0")}
        nc.sync.dma_start(kw[0][:], wk_d[0])
        xt = []
        for k in range(KC):
            t_ = xt_pool.tile([128, T], BF16, tag="xt", name=f"xt{k}")
            nc.sync.dma_start(t_[:, 0:512], xt_d[k][:, 0:512])
            xt.append(t_)
        qw = {0: wqk_pool.tile([128, C], BF16, tag="wqk", name="qw0")}
        nc.sync.dma_start(qw[0][:], wq_d[0])
        for k in range(KC):
            nc.sync.dma_start(xt[k][:, 512:T], xt_d[k][:, 512:T])
        wv_sb = []
        for k in range(KC):
            wvt = wv_pool.tile([128, GC], BF16, tag="wv", name=f"wv{k}")
            nc.sync.dma_start(wvt[:], wv_d[k])
            wv_sb.append(wvt)
        bias_t = misc_pool.tile([128, 8], F32, tag="bias", name="bias_t")
        nc.sync.dma_start(bias_t[:], bias_d[:])
        bias_sb = [bias_t[:, m:m + 1] for m in range(8)]

        def emit_warm(n_warm, wps=None):
            # p-state keep-alive: all-ones matmuls, never read
            if wps is None:
                wps = ps_small.tile([128, 512], F32, tag="ps", name="warm")
            for _ in range(n_warm):
                nc.tensor.matmul(
                    wps[:], ones_sb[:, 16:144], ones_sb[:, 16:528],
                    start=True, stop=True,
                )
            return wps

        emit_warm(16)

        # Per head the V block is 128 wide: ones in col 0 (PV psum row 0
        # = the denominator Z), v dims in cols 64:128 so the PV output
        # rows sit at the 64-aligned upper half and the normalization
        # multiply can read the psum directly.
        va = []
        for m in range(KC):
            vt = v_pool.tile([128, NHL * 128], BF16, tag="vaug", name=f"va{m}")
            view = vt[:].rearrange("p (h e) -> p h e", e=128)
            nc.vector.tensor_copy(
                view[:, :, 0:64],
                ones_sb[:, 0:NHL * 64].rearrange("p (h e) -> p h e", e=64),
            )
            va.append(vt)

        ot = []
        for k in range(KCL):
            o_ = ot_pool.tile([128, T], BF16, tag="ot", name=f"ot{k}")
            ot.append(o_)

        part = []
        for m in range(8):
            p_ = part_pool.tile([128, T], OBF16, tag="part", name=f"part{m}")
            part.append(p_)

        def emit_proj(which, p, wt, dst_pool, warm_between=False):
            d_ = dst_pool.tile([128, T], BF16, tag=which, name=f"{which}{p}")
            wps = emit_warm(0) if warm_between else None
            for n in range(2):
                ps = ps_small.tile([128, 512], F32, tag="ps",
                                   name=f"{which}{p}{n}")
                for k in range(KC):
                    nc.tensor.matmul(
                        ps[:], wt[:, k * 128:(k + 1) * 128],
                        xt[k][:, n * 512:(n + 1) * 512],
                        start=(k == 0), stop=(k == KC - 1),
                    )
                    if warm_between and n == 0 and k < 6:
                        emit_warm(1, wps)
                nc.vector.tensor_copy(d_[:, n * 512:(n + 1) * 512], ps[:])
            return d_

        def proj_gen(which, p, wt, dst_pool, out):
            # generator form of emit_proj: one matmul per next(); the
            # destination tile is stored in out[0] up-front.
            d_ = dst_pool.tile([128, T], BF16, tag=which, name=f"{which}{p}")
            out[0] = d_
            for n in range(2):
                ps = ps_small.tile([128, 512], F32, tag="ps",
                                   name=f"{which}{p}{n}")
                for k in range(KC):
                    nc.tensor.matmul(
                        ps[:], wt[:, k * 128:(k + 1) * 128],
                        xt[k][:, n * 512:(n + 1) * 512],
                        start=(k == 0), stop=(k == KC - 1),
                    )
                    if k == KC - 1:
                        nc.vector.tensor_copy(
                            d_[:, n * 512:(n + 1) * 512], ps[:]
                        )
                    yield

        def emit_s(p, hh, k_, q_, filler=None, per=2):
            # S^T for local head h = 2p+hh over all queries: one
            # [128,1024] psum pair-tile per key chunk j, one exp each.
            # Each tile's matmuls wait on EXP(j-2) via the 2-buffer psum
            # rotation; `filler` (a generator emitting one matmul per
            # next()) absorbs that stall with independent work.
            po = hh * 64
            exps = []
            for j in range(KC):
                sps = ps_s.tile([128, 2 * 512], F32, tag="ps", name=f"s{p}{hh}{j}")
                for n in range(2):
                    nc.tensor.matmul(
                        sps[:, n * 512:(n + 1) * 512],
                        k_[po:po + 64, j * 128:(j + 1) * 128],
                        q_[po:po + 64, n * 512:(n + 1) * 512],
                        start=True, stop=True,
                    )
                e = exp_pool.tile([128, 2 * 512], BF16, tag="exp",
                                  name=f"e{p}{hh}{j}")
                nc.scalar.activation(e[:], sps[:], Exp, scale=SCALE)
                exps.append(e)
                if filler is not None and j < KC - 1:
                    for _ in range(per):
                        next(filler, None)
            return exps

        def pv_chain(h, n, exps):
            # PV for local head h, query half n. Psum row 0 is Z, rows
            # 64:128 the head's output dims (64-aligned, so the
            # normalization multiply writes ot directly).
            ops = ps_small.tile([128, 512], F32, tag="ps", name=f"o{h}{n}")
            for j in range(KC):
                nc.tensor.matmul(
                    ops[:], va[j][:, h * 128:(h + 1) * 128],
                    exps[j][:, n * 512:(n + 1) * 512],
                    start=(j == 0), stop=(j == KC - 1),
                )
                yield
            okc, half = divmod(h, 2)
            po = half * 64
            rt0 = r_pool.tile([1, 512], F32, tag="rsb", name=f"r{h}{n}")
            nc.vector.reciprocal_approx_fast(rt0[0:1, :], ops[0:1, :])
            rbx = rbx_pool.tile([64, 512], F32, tag="rbx", name=f"rbx{h}{n}")
            nc.gpsimd.partition_broadcast(rbx[:], rt0[0:1, :])
            nc.vector.tensor_mul(
                ot[okc][po:po + 64, n * 512:(n + 1) * 512],
                ops[64:128, :], rbx[0:64, :],
            )

        def emit_pv(h, n, exps):
            for _ in pv_chain(h, n, exps):
                pass

        yw_sb = {}

        def y_kc0_gen(ms):
            # part[m] = bias + Wp-kc0 partial (DVE tensor_scalar_add
            # broadcasts the [128,1] bias column along tokens).
            for m in ms:
                for n in range(2):
                    sl = slice(n * 512, (n + 1) * 512)
                    ps = ps_small.tile([128, 512], F32, tag="ps",
                                       name=f"y0_{m}{n}")
                    nc.tensor.matmul(
                        ps[:], yw_sb[0][:, m * 128:(m + 1) * 128],
                        ot[0][:, sl], start=True, stop=True,
                    )
                    nc.vector.tensor_scalar_add(part[m][:, sl], ps[:],
                                                bias_sb[m])
                    yield

        def y_kc12_gen(ms):
            # part[m] += (kc1 + kc2) psum chain
            for m in ms:
                for n in range(2):
                    sl = slice(n * 512, (n + 1) * 512)
                    ps = ps_small.tile([128, 512], F32, tag="ps",
                                       name=f"y12_{m}{n}")
                    for kc in (1, 2):
                        nc.tensor.matmul(
                            ps[:], yw_sb[kc][:, m * 128:(m + 1) * 128],
                            ot[kc][:, sl], start=(kc == 1), stop=(kc == 2),
                        )
                        yield
                    nc.vector.tensor_add(part[m][:, sl], ps[:],
                                         part[m][:, sl])

        def chain_gens(*gens):
            for g in gens:
                yield from g

        # ---- pair 0 head: K/Q, then S(h0)/S(h1) with the V stage -----
        # V = x @ Wv^T ([tok, chan], ones col 0) emitted one matmul per
        # next(): it runs on the other psum pool, so pulls of it absorb
        # the EXP-rotation stalls inside the S blocks.
        def v_chain_iter():
            for m in range(KC):
                ps = ps_small.tile([128, GC], F32, tag="ps", name=f"v{m}")
                for k in range(KC):
                    nc.tensor.matmul(
                        ps[:], xt[k][:, m * 128:(m + 1) * 128], wv_sb[k][:],
                        start=(k == 0), stop=(k == KC - 1),
                    )
                    if k == KC - 1:
                        view = va[m][:].rearrange("p (h e) -> p h e", e=128)
                        nc.vector.tensor_copy(
                            view[:, :, 64:128],
                            ps[:].rearrange("p (h d) -> p h d", d=64),
                        )
                    yield

        kt_ = emit_proj("kt", 0, kw[0], kt_pool, warm_between=True)
        qt_ = emit_proj("qt", 0, qw[0], qt_pool)
        vgen = v_chain_iter()
        exps_a = emit_s(0, 0, kt_, qt_, filler=vgen, per=3)
        for _ in range(16):  # keep V flowing between the two S blocks
            next(vgen, None)

        kw[1] = wqk_pool.tile([128, C], BF16, tag="wqk", name="kw1")
        nc.sync.dma_start(kw[1][:], wk_d[1])
        qw[1] = wqk_pool.tile([128, C], BF16, tag="wqk", name="qw1")
        nc.sync.dma_start(qw[1][:], wq_d[1])

        exps_b = emit_s(0, 1, kt_, qt_, filler=vgen, per=3)
        for _ in vgen:  # drain the V remainder before any PV
            pass

        # pair 0 PV region: kt1 / qt1 dense between PV chains (they
        # must complete before pair 1's S blocks read them).
        emit_pv(0, 0, exps_a)
        emit_pv(0, 1, exps_a)
        ktn = emit_proj("kt", 1, kw[1], kt_pool)
        emit_pv(1, 0, exps_b)
        qtn = emit_proj("qt", 1, qw[1], qt_pool)
        emit_pv(1, 1, exps_b)
        kt_, qt_ = ktn, qtn

        # prefetch weights for pair 2 and the first Y tiles
        kw[2] = wqk_pool.tile([128, C], BF16, tag="wqk", name="kw2")
        nc.sync.dma_start(kw[2][:], wk_d[2])
        qw[2] = wqk_pool.tile([128, C], BF16, tag="wqk", name="qw2")
        nc.sync.dma_start(qw[2][:], wq_d[2])
        for m in (0, 1):
            yw = wp_pool.tile([128, C], BF16, tag="wp", name=f"yw{m}")
            nc.sync.dma_start(yw[:], wp_d[m])
            yw_sb[m] = yw

        # ---- pairs 1-3 ----------------------------------------------
        # S blocks carry explicit filler generators; PV chains of head a
        # interleave into S_b where the proj supply runs short.
        holder = [None]

        # pair 1
        g = proj_gen("kt", 2, kw[2], kt_pool, holder)
        exps_a = emit_s(1, 0, kt_, qt_, filler=g, per=3)
        for _ in g:
            pass
        kt2 = holder[0]
        holder = [None]
        g = chain_gens(
            proj_gen("qt", 2, qw[2], qt_pool, holder),
            y_kc0_gen(range(8)),
        )
        exps_b = emit_s(1, 1, kt_, qt_, filler=g, per=4)
        for _ in g:
            pass
        qt2 = holder[0]
        emit_pv(2, 0, exps_a)
        emit_pv(2, 1, exps_a)
        emit_pv(3, 0, exps_b)
        emit_pv(3, 1, exps_b)
        kt_, qt_ = kt2, qt2

        kw[3] = wqk_pool.tile([128, C], BF16, tag="wqk", name="kw3")
        nc.sync.dma_start(kw[3][:], wk_d[3])
        qw[3] = wqk_pool.tile([128, C], BF16, tag="wqk", name="qw3")
        nc.sync.dma_start(qw[3][:], wq_d[3])
        for m in (2, 3):
            yw = wp_pool.tile([128, C], BF16, tag="wp", name=f"yw{m}")
            nc.sync.dma_start(yw[:], wp_d[m])
            yw_sb[m] = yw

        # pair 2
        holder = [None]
        g = proj_gen("kt", 3, kw[3], kt_pool, holder)
        exps_a = emit_s(2, 0, kt_, qt_, filler=g, per=3)
        for _ in g:
            pass
        kt3 = holder[0]
        holder = [None]
        g = chain_gens(
            proj_gen("qt", 3, qw[3], qt_pool, holder),
            pv_chain(4, 0, exps_a),
            pv_chain(4, 1, exps_a),
        )
        exps_b = emit_s(2, 1, kt_, qt_, filler=g, per=4)
        for _ in g:
            pass
        qt3 = holder[0]
        emit_pv(5, 0, exps_b)
        emit_pv(5, 1, exps_b)
        kt_, qt_ = kt3, qt3

        # pair 3
        g = y_kc12_gen(range(0, 4))
        exps_a = emit_s(3, 0, kt_, qt_, filler=g, per=3)
        for _ in g:
            pass
        g = chain_gens(
            y_kc12_gen(range(4, 6)),
            pv_chain(6, 0, exps_a),
            pv_chain(6, 1, exps_a),
        )
        exps_b = emit_s(3, 1, kt_, qt_, filler=g, per=4)
        for _ in g:
            pass
        emit_pv(7, 0, exps_b)
        for _ in y_kc12_gen(range(6, 7)):
            pass
        emit_pv(7, 1, exps_b)
        for _ in y_kc12_gen(range(7, 8)):
            pass

        # ---- output projection: kc3 + one DVE add per tile ----------
        for m in range(8):
            ps = ps_s.tile([128, 2 * 512], F32, tag="ps", name=f"y3_{m}")
            for n in range(2):
                sl = slice(n * 512, (n + 1) * 512)
                nc.tensor.matmul(
                    ps[:, sl], yw_sb[3][:, m * 128:(m + 1) * 128],
                    ot[3][:, sl], start=True, stop=True,
                )
            ysb = y_pool.tile([128, T], OBF16, tag="ysb", name=f"ysb{m}")
            nc.vector.tensor_add(ysb[:], ps[:], part[m][:])
            nc.sync.dma_start(yt_d[m][:], ysb[:])

    nc.compile()
    return nc


def _get_program():
    global _PROG
    if _PROG is None:
        _PROG = _build_program()
    return _PROG


def _prep_inputs(x, Wqkv, Wproj, bproj):
    """Host-side shard prep: per-core input maps."""
    import ml_dtypes

    bf16 = np.float32 if USE_F32R else ml_dtypes.bfloat16
    x = np.asarray(x, dtype=np.float32)
    Wqkv = np.asarray(Wqkv, dtype=np.float32)
    Wproj = np.asarray(Wproj, dtype=np.float32)
    bproj = np.asarray(bproj, dtype=np.float32)

    def stat_cols(wT_slice):
        # [C, GC] (in chan, local out) -> [KCL, 128, C] stationary tiles:
        # tile[p][part, kc*128+m] = wT_slice[kc*128+part, p*128+m]
        return np.ascontiguousarray(
            wT_slice.reshape(KC, 128, KCL, 128)
            .transpose(2, 1, 0, 3)
            .reshape(KCL, 128, C)
        ).astype(bf16)

    bias = np.ascontiguousarray(bproj.reshape(8, 128).T)
    zbias = np.zeros_like(bias)

    per_g = []
    for g in range(2):
        lo = GC * g
        wq = stat_cols(Wqkv[0 * C + lo:0 * C + lo + GC].T)
        wk = stat_cols(Wqkv[1 * C + lo:1 * C + lo + GC].T)
        wv = np.ascontiguousarray(
            Wqkv[2 * C + lo:2 * C + lo + GC].T.reshape(KC, 128, GC)
        ).astype(bf16)
        # yw[kc][part, m*128+d] = Wproj^T[lo + kc*128 + part, m*128+d]
        wp = np.ascontiguousarray(
            Wproj.T[lo:lo + GC].reshape(KCL, 128, C)
        ).astype(bf16)
        per_g.append((wq, wk, wv, wp))

    in_maps = []
    for i in range(8):
        b, g = divmod(i, 2)
        wq, wk, wv, wp = per_g[g]
        xt = np.ascontiguousarray(x[b].T.reshape(KC, 128, T)).astype(bf16)
        in_maps.append(
            {
                "xt": xt, "wq": wq, "wk": wk, "wv": wv, "wp": wp,
                "bias": bias if g == 0 else zbias,
            }
        )
    return in_maps


def _assemble(results, x_dtype):
    out = np.empty((B, T, C), dtype=np.float32)
    for b in range(B):
        yt = results[2 * b]["yt"].reshape(C, T).astype(np.float32)
        yt = yt + results[2 * b + 1]["yt"].reshape(C, T)
        out[b] = yt.T
    return out.astype(x_dtype, copy=False)


def run(inputs, trace=False, **spmd_kwargs):
    """Shared entry for kernel() and test harnesses (trace for profiling)."""
    from concourse.bass_utils import run_bass_kernel_spmd

    nc = _get_program()
    in_maps = _prep_inputs(**inputs)
    res = run_bass_kernel_spmd(
        nc, in_maps, list(range(8)), trace=trace, **spmd_kwargs
    )
    out = _assemble(res.results, np.asarray(inputs["x"]).dtype)
    return out, res


def kernel(x, Wqkv, Wproj, bproj):
    out, _ = run(dict(x=x, Wqkv=Wqkv, Wproj=Wproj, bproj=bproj))
    return out
